# revision 21
# baseline (speedup 1.0000x reference)
"""BEVSampling Trainium2 kernel.

Strategy: data-parallel over the Q=10000 BEV queries across 8 cores
(1250 queries = 10368 padded points per core). Per core, on device:
  - denormalize reference points + offsets (T4), project into all 6
    cameras with one small matmul per 128-point tile,
  - compute u/v/valid/bilinear weights/patch indices with wide DVE ops,
  - gather 2x2 bilinear patches (one 1KB descriptor per (cam,point))
    from a host-prepped pixel-major fp16 feature table via indirect DMA,
  - weighted tap-combine on DVE, PE-transpose to feature-major,
  - positional MLP + 3-layer per-query MLP entirely on TensorE,
  - outputs: out[C, 1250] fp32 and u/v/valid dumps for cam_uv.
Host only shards/relayouts inputs and concatenates outputs.
"""
import numpy as np

import concourse.bass as bass
import concourse.mybir as mybir
import concourse.tile as tile
from concourse import bass_utils
from concourse.masks import make_identity
from concourse.vector_clock import ScopedClock

F32 = mybir.dt.float32
F32R = mybir.dt.float32r
F16 = mybir.dt.float16
I32 = mybir.dt.int32
AOT = mybir.AluOpType
ACTF = mybir.ActivationFunctionType
AXT = mybir.AxisListType

PC_LO = np.array([-50.0, -50.0, -5.0], dtype=np.float32)
PC_SPAN = np.array([100.0, 100.0, 8.0], dtype=np.float32)
EPS = 1e-5
N, C, Hf, Wf, P = 6, 128, 28, 60, 8
NCORES = 8
QG = 10000                 # global queries
QC = 1250                  # real queries per core
MQ = 1296                  # padded queries per core (27*48)
M = MQ * 8                 # 10368 points per core
NT = M // 128              # 81 tiles of 128 points
NG, GM = 27, 384           # groups for the (g,j)-partition layout
R2 = N * Hf * Wf + 61      # table rows (padded)
NQCH = 3                   # query chunks in conv MLP
QCH = MQ // NQCH           # 432
TB = 3                     # m-tiles per indirect-DMA batch
NS = 4                     # camera slots per point after compaction


def _patch_tile_drain():
    """walrus in this container allows only 1 sync wait per Drain: spread
    the Tile epilogue drain's waits across multiple drain instructions."""
    if getattr(tile.TileContext, "_drain_patched", False):
        return

    def _drain_and_barrier(self, tick_clock, wait_clock):
        drain_inst = self.nc.sync.drain()
        wait_clock.add_sem_waits(
            drain_inst.ins, ScopedClock({None: tick_clock.global_clock})
        )
        si = drain_inst.ins.sync_info
        w = list(si.on_wait or []) if si is not None else []
        if len(w) > 1:
            si.on_wait = w[:1]
            for sw in w[1:]:
                d2 = self.nc.sync.drain()
                d2.ins.sync_info = mybir.SyncInfo(on_wait=[sw], on_update=[])
        self.nc.all_engine_barrier()
        assert self.sems is not None
        popped = self.nc._tile_sem_poison_stack.pop()
        assert popped is self._sem_poison
        self.nc.clear_and_free_semaphores(list(self.sems.allocated().values()))
        self.nc.all_engine_barrier()

    tile.TileContext._drain_and_barrier = _drain_and_barrier
    tile.TileContext._drain_patched = True


def _patch_compile_bir():
    """This walrus build accepts at most ONE sync wait per instruction.
    Tile emits several. Rewrite the serialized BIR before compiling:
    hoist extra waits onto same-engine NoOps inserted just before."""
    import json
    import concourse.bass2jax as b2j
    if getattr(b2j, "_split_wait_patched", False):
        return
    orig = b2j.compile_bir_kernel

    def _split(bir_bytes):
        j = json.loads(bir_bytes)
        ctr = [0]
        for fn in j.get("functions", []):
            for blk in fn.get("blocks", []):
                insts = blk.get("instructions", [])
                out = []
                for inst in insts:
                    si = inst.get("sync_info")
                    if si and si.get("on_wait") and len(si["on_wait"]) > 1:
                        waits = si["on_wait"]
                        for wv in waits[:-1]:
                            ctr[0] += 1
                            out.append({
                                "opcode": "NoOp",
                                "engine": inst.get("engine", "SP"),
                                "name": f"nopw-{ctr[0]}",
                                "ins": [],
                                "outs": [],
                                "debug": inst.get("debug", 0),
                                "sync_info": {"on_update": [],
                                              "on_wait": [wv]},
                            })
                        si["on_wait"] = [waits[-1]]
                    out.append(inst)
                blk["instructions"] = out
        return json.dumps(j).encode()

    def wrapped(bir_bytes, *a, **k):
        return orig(_split(bir_bytes), *a, **k)

    b2j.compile_bir_kernel = wrapped
    b2j._split_wait_patched = True


def build_program():
    _patch_tile_drain()
    _patch_compile_bir()
    nc = bass.Bass("TRN2", target_bir_lowering=False)

    di = {}
    def inp(name, shape, dt):
        di[name] = nc.dram_tensor(name, shape, dt, kind="ExternalInput")
        return di[name]

    rp_w4 = inp("rp_w4", [128, GM], F32)
    off_w4 = inp("off_w4", [128, GM], F32)
    scale_vec = inp("scale_vec", [128, 1], F32)
    bias_vec = inp("bias_vec", [128, 1], F32)
    s2_vec = inp("s2_vec", [128, 1], F32)
    b2n_vec = inp("b2n_vec", [128, 1], F32)
    L_in = inp("L_in", [4, 24], F32)
    T2 = inp("T2", [R2, 512], F16)
    w1_in = inp("w1_in", [3, 256], F16)
    b1_in = inp("b1_in", [128, 2], F32)
    w2_in = inp("w2_in", [128, 2, 128], F16)
    b2_in = inp("b2_in", [128, 1], F32)
    cw1_in = inp("cw1_in", [128, 8, 512], F16)
    cb1_in = inp("cb1_in", [128, 4], F32)
    cw2_in = inp("cw2_in", [128, 4, 512], F16)
    cb2_in = inp("cb2_in", [128, 4], F32)
    cw3_in = inp("cw3_in", [128, 4, 128], F16)
    cb3_in = inp("cb3_in", [128, 1], F32)

    t4_dram = nc.dram_tensor("t4_dram", [128, GM], F32)
    t4n_dram = nc.dram_tensor("t4n_dram", [128, GM], F32)
    out_d = nc.dram_tensor("out_d", [128, QC], F32, kind="ExternalOutput")
    u_d = nc.dram_tensor("u_d", [128, NT * 6], F32, kind="ExternalOutput")
    v_d = nc.dram_tensor("v_d", [128, NT * 6], F32, kind="ExternalOutput")
    s_d = nc.dram_tensor("s_d", [128, NT * 6], F32, kind="ExternalOutput")

    W = NT * 6  # 486

    with tile.TileContext(nc) as tc:
        with tc.tile_pool(name="const", bufs=1) as cpool, \
             tc.tile_pool(name="keep", bufs=1) as kpool, \
             tc.tile_pool(name="psA", bufs=2, space="PSUM") as psA, \
             tc.tile_pool(name="psB", bufs=2, space="PSUM") as psB:

            # ---- constant / input loads ----
            def load(dram, shape, dt, name):
                t = cpool.tile(shape, dt, tag=name, name=name)
                nc.sync.dma_start(t[:], dram[:])
                return t
            scv = load(scale_vec, [128, 1], F32, "scv")
            biv = load(bias_vec, [128, 1], F32, "biv")
            s2v = load(s2_vec, [128, 1], F32, "s2v")
            b2nv = load(b2n_vec, [128, 1], F32, "b2nv")
            Lsb = load(L_in, [4, 24], F32, "Lsb")
            w1s = load(w1_in, [3, 256], F16, "w1s")
            b1s = load(b1_in, [128, 2], F32, "b1s")
            w2s = load(w2_in, [128, 2, 128], F16, "w2s")
            b2s = load(b2_in, [128, 1], F32, "b2s")
            cw1s = load(cw1_in, [128, 8, 512], F16, "cw1s")
            cb1s = load(cb1_in, [128, 4], F32, "cb1s")
            cw2s = load(cw2_in, [128, 4, 512], F16, "cw2s")
            cb2s = load(cb2_in, [128, 4], F32, "cb2s")
            cw3s = load(cw3_in, [128, 4, 128], F16, "cw3s")
            cb3s = load(cb3_in, [128, 1], F32, "cb3s")

            ident = cpool.tile([128, 128], F16, tag="ident")
            make_identity(nc, ident[:])

            # persistent across phases
            xT = kpool.tile([128, M], F16, tag="xT")
            W4h = kpool.tile([128, NT, NS, 4], F16, tag="W4h")
            idx32 = kpool.tile([128, NT, NS], I32, tag="idx32")
            outs = kpool.tile([128, MQ], F32, tag="outs")

            CH = 1152               # m per chunk (3 groups, 9 m-tiles)
            NCH = M // CH           # 9

            with tc.tile_pool(name="ph1", bufs=1) as wpool, \
                 tc.tile_pool(name="ph1d", bufs=2) as dpool:
                rp4 = wpool.tile([128, GM], F32, tag="rp4")
                off4 = wpool.tile([128, GM], F32, tag="off4")
                nc.sync.dma_start(rp4[:], rp_w4[:])
                nc.sync.dma_start(off4[:], off_w4[:])
                iota_n = wpool.tile([128, NT, 6], F32, tag="iota_n")
                nc.gpsimd.iota(iota_n[:], pattern=[[0, NT], [1680, 6]], base=0,
                               channel_multiplier=0,
                               allow_small_or_imprecise_dtypes=True)

                # Phase A: T4 (world homog points), T4n (renormalized)
                T4 = wpool.tile([128, GM], F32, tag="T4")
                nc.scalar.activation(T4[:], rp4[:], ACTF.Identity,
                                     bias=biv[:, :1], scale=scv[:, :1])
                nc.vector.tensor_add(T4[:], T4[:], off4[:])
                T4n = wpool.tile([128, GM], F32, tag="T4n")
                nc.scalar.activation(T4n[:], T4[:], ACTF.Identity,
                                     bias=b2nv[:, :1], scale=s2v[:, :1])
                nc.sync.dma_start(t4_dram[:], T4[:])
                nc.sync.dma_start(t4n_dram[:], T4n[:])

                # per-chunk: shuffle to [4, CH], project, pos-MLP into xT
                proj = wpool.tile([128, NT, 24], F32, tag="proj")
                for c in range(NCH):
                    g0 = c * 3
                    t4c = dpool.tile([4, CH], F32, tag="t4c")
                    t4nc = dpool.tile([4, CH], F16, tag="t4nc")
                    src = t4_dram[g0 * 4:(g0 + 3) * 4, :].rearrange(
                        "(g j) mm -> j g mm", j=4)
                    nc.sync.dma_start(
                        t4c[:].rearrange("j (g mm) -> j g mm", g=3), src)
                    srcn = t4n_dram[g0 * 4:(g0 + 3) * 4, :].rearrange(
                        "(g j) mm -> j g mm", j=4)
                    nc.gpsimd.dma_start(
                        t4nc[:].rearrange("j (g mm) -> j g mm", g=3), srcn)

                    # projection: 9 m-tiles -> psum -> proj
                    pst = psA.tile([128, 216], F32, tag="pp")
                    for tt in range(9):
                        t = c * 9 + tt
                        nc.tensor.matmul(
                            pst[:, tt * 24:(tt + 1) * 24],
                            t4c[:, tt * 128:(tt + 1) * 128],
                            Lsb[:4, :],
                            start=True, stop=True)
                    nc.vector.tensor_copy(proj[:, c * 9:(c + 1) * 9, :], pst[:])

                    # pos MLP for the 3 groups of this chunk -> xT
                    for gg in range(3):
                        rhs = t4nc[:3, gg * GM:(gg + 1) * GM]
                        h1g = dpool.tile([128, 2, GM], F16, tag="h1g")
                        for h in range(2):
                            psh = psB.tile([128, GM], F32, tag="psh")
                            nc.tensor.matmul(
                                psh[:],
                                w1s[:3, h * 128:(h + 1) * 128],
                                rhs,
                                start=True, stop=True)
                            nc.scalar.activation(h1g[:, h], psh[:], ACTF.Relu,
                                                 bias=b1s[:, h:h + 1], scale=1.0)
                        ppos = psB.tile([128, GM], F32, tag="ppos")
                        for h in range(2):
                            nc.tensor.matmul(ppos[:], w2s[:, h], h1g[:, h],
                                             start=(h == 0), stop=(h == 1))
                        g = g0 + gg
                        nc.vector.tensor_copy(xT[:, g * GM:(g + 1) * GM],
                                              ppos[:])

                # Phase B2: u/v/valid/weights/indices (wide ops over all tiles)
                xv = proj[:, :, 0:24:4]
                yv = proj[:, :, 1:24:4]
                zv = proj[:, :, 2:24:4]
                def wt(name, dt=F32):
                    return wpool.tile([128, NT, 6], dt, tag=name, name=name)
                zd = wt("zd"); r0 = wt("r0"); e = wt("e"); r = wt("r")
                xr = wt("xr"); yr = wt("yr")
                nc.vector.tensor_scalar_max(zd[:], zv, EPS)
                nc.vector.reciprocal(r0[:], zd[:])
                nc.vector.tensor_mul(e[:], zd[:], r0[:])
                nc.scalar.activation(e[:], e[:], ACTF.Copy, bias=2.0, scale=-1.0)
                nc.vector.tensor_mul(r[:], r0[:], e[:])
                nc.vector.tensor_mul(xr[:], xv, r[:])
                nc.vector.tensor_mul(yr[:], yv, r[:])

                Usb = wpool.tile([128, NT, 6], F32, tag="Usb")
                Vsb = wpool.tile([128, NT, 6], F32, tag="Vsb")
                Ssb = wpool.tile([128, NT, 6], F32, tag="Ssb")
                nc.vector.tensor_scalar_mul(Usb[:], xr[:], 1.0 / 480.0)
                nc.vector.tensor_scalar_mul(Vsb[:], yr[:], 1.0 / 224.0)

                xi = wt("xi"); yi = wt("yi")
                nc.scalar.activation(xi[:], xr[:], ACTF.Copy, bias=-0.5, scale=0.125)
                nc.scalar.activation(yi[:], yr[:], ACTF.Copy, bias=-0.5, scale=0.125)

                ta = wt("ta"); tb = wt("tb"); tcm = wt("tcm")
                nc.vector.tensor_scalar(ta[:], zv, EPS, None, op0=AOT.is_gt)
                nc.vector.tensor_scalar(tb[:], Usb[:], 0.0, None, op0=AOT.is_gt)
                nc.vector.tensor_scalar(tcm[:], Usb[:], 1.0, None, op0=AOT.is_lt)
                nc.vector.tensor_mul(ta[:], ta[:], tb[:])
                nc.vector.tensor_mul(ta[:], ta[:], tcm[:])
                nc.vector.tensor_scalar(tb[:], Vsb[:], 0.0, None, op0=AOT.is_gt)
                nc.vector.tensor_scalar(tcm[:], Vsb[:], 1.0, None, op0=AOT.is_lt)
                nc.vector.tensor_mul(ta[:], ta[:], tb[:])
                nc.vector.tensor_mul(Ssb[:], ta[:], tcm[:])      # valid

                # floor(xi) -> x0 ; floor(yi) -> y0
                ic = wpool.tile([128, NT, 6], I32, tag="ic")
                x0 = wt("x0"); y0 = wt("y0"); wx = wt("wx"); wy = wt("wy")
                nc.vector.tensor_copy(ic[:], xi[:])
                nc.vector.tensor_copy(x0[:], ic[:])
                nc.vector.tensor_tensor(tb[:], x0[:], xi[:], op=AOT.is_gt)
                nc.vector.tensor_sub(x0[:], x0[:], tb[:])
                nc.vector.tensor_copy(ic[:], yi[:])
                nc.vector.tensor_copy(y0[:], ic[:])
                nc.vector.tensor_tensor(tb[:], y0[:], yi[:], op=AOT.is_gt)
                nc.vector.tensor_sub(y0[:], y0[:], tb[:])
                nc.vector.tensor_sub(wx[:], xi[:], x0[:])
                nc.vector.tensor_sub(wy[:], yi[:], y0[:])

                # a0/a1 (x taps), b0/b1 (y taps)
                a0 = wt("a0"); a1 = wt("a1"); b0 = wt("b0"); b1 = wt("b1")
                omw = wt("omw")
                nc.vector.tensor_scalar(ta[:], x0[:], 0.0, None, op0=AOT.is_ge)
                nc.vector.tensor_scalar(tb[:], x0[:], 59.0, None, op0=AOT.is_le)
                nc.vector.tensor_mul(tb[:], ta[:], tb[:])
                nc.scalar.activation(omw[:], wx[:], ACTF.Copy, bias=1.0, scale=-1.0)
                nc.vector.tensor_mul(a0[:], tb[:], omw[:])
                nc.vector.tensor_scalar(tcm[:], x0[:], -1.0, None, op0=AOT.is_equal)
                nc.vector.tensor_mul(tcm[:], tcm[:], wx[:])
                nc.vector.tensor_add(a0[:], a0[:], tcm[:])
                nc.vector.tensor_scalar(tb[:], x0[:], 58.0, None, op0=AOT.is_le)
                nc.vector.tensor_mul(tb[:], ta[:], tb[:])
                nc.vector.tensor_mul(a1[:], tb[:], wx[:])
                nc.vector.tensor_scalar(ta[:], y0[:], 0.0, None, op0=AOT.is_ge)
                nc.vector.tensor_scalar(tb[:], y0[:], 27.0, None, op0=AOT.is_le)
                nc.vector.tensor_mul(tb[:], ta[:], tb[:])
                nc.scalar.activation(omw[:], wy[:], ACTF.Copy, bias=1.0, scale=-1.0)
                nc.vector.tensor_mul(b0[:], tb[:], omw[:])
                nc.vector.tensor_scalar(tcm[:], y0[:], -1.0, None, op0=AOT.is_equal)
                nc.vector.tensor_mul(tcm[:], tcm[:], wy[:])
                nc.vector.tensor_add(b0[:], b0[:], tcm[:])
                nc.vector.tensor_scalar(tb[:], y0[:], 26.0, None, op0=AOT.is_le)
                nc.vector.tensor_mul(tb[:], ta[:], tb[:])
                nc.vector.tensor_mul(b1[:], tb[:], wy[:])
                nc.vector.tensor_mul(a0[:], a0[:], Ssb[:])
                nc.vector.tensor_mul(a1[:], a1[:], Ssb[:])

                # weights fp16, k = (b0a0, b1a0, b0a1, b1a1)
                W4f = wpool.tile([128, NT, 6, 4], F32, tag="W4f")
                nc.vector.tensor_mul(W4f[:, :, :, 0], b0[:], a0[:])
                nc.vector.tensor_mul(W4f[:, :, :, 1], b0[:], a1[:])
                nc.vector.tensor_mul(W4f[:, :, :, 2], b1[:], a0[:])
                nc.vector.tensor_mul(W4f[:, :, :, 3], b1[:], a1[:])
                # patch index = clip(y0,0,27)*60 + clip(x0,0,59) + n*1680
                nc.vector.tensor_scalar(ta[:], x0[:], 0.0, 59.0,
                                        op0=AOT.max, op1=AOT.min)
                nc.vector.tensor_scalar(tb[:], y0[:], 0.0, 27.0,
                                        op0=AOT.max, op1=AOT.min)
                nc.vector.tensor_scalar_mul(tb[:], tb[:], 60.0)
                nc.vector.tensor_add(ta[:], ta[:], tb[:])
                nc.vector.tensor_add(ta[:], ta[:], iota_n[:])

                # per-point camera compaction into NS slots
                cum = wpool.tile([128, NT, 6], F32, tag="cum")
                nc.vector.memset(cum[:, :, 0], 0.0)
                for n in range(1, 6):
                    nc.vector.tensor_add(cum[:, :, n], cum[:, :, n - 1],
                                         Ssb[:, :, n - 1])
                sel = wpool.tile([128, NT, 6], F32, tag="sel")
                msk = wpool.tile([128, NT, 6], F32, tag="msk")
                red = wpool.tile([128, NT], F32, tag="red")
                W4S = wpool.tile([128, NT, NS, 4], F32, tag="W4S")
                for ss in range(NS):
                    nc.vector.tensor_scalar(sel[:], cum[:], float(ss), None,
                                            op0=AOT.is_equal)
                    nc.vector.tensor_mul(sel[:], sel[:], Ssb[:])
                    nc.vector.tensor_mul(msk[:], sel[:], ta[:])
                    nc.vector.tensor_reduce(red[:], msk[:], axis=AXT.X,
                                            op=AOT.add)
                    nc.vector.tensor_copy(idx32[:, :, ss], red[:])
                    for k in range(4):
                        nc.vector.tensor_mul(msk[:], sel[:], W4f[:, :, :, k])
                        nc.vector.tensor_reduce(W4S[:, :, ss, k], msk[:],
                                                axis=AXT.X, op=AOT.add)
                nc.vector.tensor_copy(W4h[:], W4S[:])

                nc.sync.dma_start(u_d[:],
                                  Usb[:].rearrange("p t n -> p (t n)"))
                nc.sync.dma_start(v_d[:],
                                  Vsb[:].rearrange("p t n -> p (t n)"))
                nc.sync.dma_start(s_d[:],
                                  Ssb[:].rearrange("p t n -> p (t n)"))

            # ---- Phase C+D: gather + combine + transpose (adds into xT) ----
            with tc.tile_pool(name="gbuf", bufs=3) as gpool, \
                 tc.tile_pool(name="small", bufs=2) as spool:
                for t in range(NT):
                    G4 = gpool.tile([128, NS, 4, 128], F16, tag="G4")
                    for ss in range(NS):
                        nc.gpsimd.indirect_dma_start(
                            out=G4[:, ss].rearrange("p a b -> p (a b)"),
                            out_offset=None,
                            in_=T2[:, :],
                            in_offset=bass.IndirectOffsetOnAxis(
                                ap=idx32[:, t, ss:ss + 1], axis=0),
                        )
                    Gw = spool.tile([128, NS * 4, 128], F16, tag="Gw")
                    wbc = W4h[:, t].rearrange(
                        "p n k -> p (n k)").to_broadcast([128, NS * 4, 128])
                    nc.vector.tensor_tensor(
                        Gw[:], G4[:].rearrange("p n k c -> p (n k) c"), wbc,
                        op=AOT.mult)
                    r1 = spool.tile([128, 8, 128], F16, tag="r1")
                    nc.vector.tensor_add(r1[:], Gw[:, 0:8], Gw[:, 8:16])
                    r2 = spool.tile([128, 4, 128], F16, tag="r2")
                    nc.vector.tensor_add(r2[:], r1[:, 0:4], r1[:, 4:8])
                    r3 = spool.tile([128, 2, 128], F16, tag="r3")
                    nc.vector.tensor_add(r3[:], r2[:, 0:2], r2[:, 2:4])
                    samp = spool.tile([128, 128], F16, tag="samp")
                    nc.vector.tensor_add(samp[:], r3[:, 0], r3[:, 1])
                    ptr = psA.tile([128, 128], F16, tag="pp")
                    nc.tensor.transpose(ptr[:], samp[:], ident[:])
                    xs = xT[:, t * 128:(t + 1) * 128]
                    nc.vector.tensor_add(xs, xs, ptr[:])

            # + b2 (per-channel)
            nc.vector.tensor_scalar_add(xT[:], xT[:], b2s[:, :1])

            # ---- Phase F: conv MLP ----
            with tc.tile_pool(name="mlp", bufs=2) as mpool:
                xTv = xT.rearrange("c (q p) -> c q p", p=8)
                for qc in range(NQCH):
                    q0 = qc * QCH
                    h1c = mpool.tile([128, 4, QCH], F16, tag="h1c")
                    for mt in range(4):
                        ph = psA.tile([128, QCH], F32, tag="ph")
                        for p in range(8):
                            nc.tensor.matmul(
                                ph[:], cw1s[:, p, mt * 128:(mt + 1) * 128],
                                xTv[:, q0:q0 + QCH, p],
                                start=(p == 0), stop=(p == 7))
                        nc.scalar.activation(h1c[:, mt], ph[:], ACTF.Gelu,
                                             bias=cb1s[:, mt:mt + 1], scale=1.0)
                    h2c = mpool.tile([128, 4, QCH], F16, tag="h2c")
                    for mt in range(4):
                        ph = psA.tile([128, QCH], F32, tag="ph")
                        for k in range(4):
                            nc.tensor.matmul(
                                ph[:], cw2s[:, k, mt * 128:(mt + 1) * 128],
                                h1c[:, k],
                                start=(k == 0), stop=(k == 3))
                        nc.scalar.activation(h2c[:, mt], ph[:], ACTF.Gelu,
                                             bias=cb2s[:, mt:mt + 1], scale=1.0)
                    ph = psA.tile([128, QCH], F32, tag="ph")
                    for k in range(4):
                        nc.tensor.matmul(ph[:], cw3s[:, k], h2c[:, k],
                                         start=(k == 0), stop=(k == 3))
                    nc.scalar.activation(outs[:, q0:q0 + QCH], ph[:],
                                         ACTF.Identity, bias=cb3s[:, :1],
                                         scale=1.0)

            nc.sync.dma_start(out_d[:], outs[:, :QC])

    return nc


def host_prep(inputs):
    """Build the 8 per-core input dicts (pure relayout / fp16 cast)."""
    rp = np.ascontiguousarray(inputs["reference_points"][0].reshape(QG, 3))
    off = np.ascontiguousarray(inputs["sampling_offset"][0].reshape(QG, 8, 3))
    feats = inputs["feats"][0]
    l2i = inputs["lidar2img"][0]

    # shared tensors
    scale_vec = np.zeros((128, 1), np.float32)
    bias_vec = np.zeros((128, 1), np.float32)
    s2_vec = np.zeros((128, 1), np.float32)
    b2n_vec = np.zeros((128, 1), np.float32)
    for g in range(NG):
        for j in range(3):
            r = g * 4 + j
            scale_vec[r] = PC_SPAN[j]
            bias_vec[r] = PC_LO[j]
            s2_vec[r] = np.float32(1.0) / PC_SPAN[j]
            b2n_vec[r] = -PC_LO[j] / PC_SPAN[j]
        scale_vec[g * 4 + 3] = 0.0
        bias_vec[g * 4 + 3] = 1.0

    L_in = np.zeros((4, 24), np.float32)
    for n in range(N):
        for i in range(4):
            L_in[:, n * 4 + i] = l2i[n, i, :]

    T2 = np.zeros((R2, 512), np.float16)
    fpix = feats.astype(np.float16).transpose(0, 2, 3, 1)    # [N,Hf,Wf,C]
    ent = np.zeros((N, Hf, Wf, 4, 128), np.float16)
    ent[:, :, :, 0] = fpix
    ent[:, :, :Wf - 1, 1] = fpix[:, :, 1:]
    ent[:, :Hf - 1, :, 2] = fpix[:, 1:, :]
    ent[:, :Hf - 1, :Wf - 1, 3] = fpix[:, 1:, 1:]
    T2[:N * Hf * Wf] = ent.reshape(-1, 512)

    w1_in = np.ascontiguousarray(inputs["w1"]).astype(np.float16)
    b1_in = np.zeros((128, 2), np.float32)
    b1_in[:, 0] = inputs["b1"][:128]
    b1_in[:, 1] = inputs["b1"][128:]
    w2_in = np.ascontiguousarray(
        inputs["w2"].reshape(2, 128, 128).transpose(1, 0, 2)).astype(np.float16)
    b2_in = np.ascontiguousarray(inputs["b2"].reshape(128, 1)).astype(np.float32)
    cw1_in = np.ascontiguousarray(
        inputs["cw1"].reshape(8, 128, 512).transpose(1, 0, 2)).astype(np.float16)
    cb1_in = np.ascontiguousarray(
        inputs["cb1"].reshape(4, 128).T).astype(np.float32)
    cw2_in = np.ascontiguousarray(
        inputs["cw2"].reshape(4, 128, 512).transpose(1, 0, 2)).astype(np.float16)
    cb2_in = np.ascontiguousarray(
        inputs["cb2"].reshape(4, 128).T).astype(np.float32)
    cw3_in = np.ascontiguousarray(
        inputs["cw3"].reshape(4, 128, 128).transpose(1, 0, 2)).astype(np.float16)
    cb3_in = np.ascontiguousarray(inputs["cb3"].reshape(128, 1)).astype(np.float32)

    shared = dict(scale_vec=scale_vec, bias_vec=bias_vec, s2_vec=s2_vec,
                  b2n_vec=b2n_vec, L_in=L_in, T2=T2, w1_in=w1_in, b1_in=b1_in,
                  w2_in=w2_in, b2_in=b2_in, cw1_in=cw1_in, cb1_in=cb1_in,
                  cw2_in=cw2_in, cb2_in=cb2_in, cw3_in=cw3_in, cb3_in=cb3_in)

    in_maps = []
    marr = np.arange(M)
    for core in range(NCORES):
        qidx = np.minimum(core * QC + marr // 8, QG - 1)
        pidx = marr % 8
        rp_w4 = np.zeros((128, GM), np.float32)
        off_w4 = np.zeros((128, GM), np.float32)
        rpm = rp[qidx]                                       # [M, 3]
        offm = off[qidx, pidx]                               # [M, 3]
        rpv = rpm.reshape(NG, GM, 3)
        offv = offm.reshape(NG, GM, 3)
        for g in range(NG):
            for j in range(3):
                rp_w4[g * 4 + j, :] = rpv[g, :, j]
                off_w4[g * 4 + j, :] = offv[g, :, j]
        m = dict(shared)
        m["rp_w4"] = rp_w4
        m["off_w4"] = off_w4
        in_maps.append(m)
    return in_maps


_NC_CACHE = {}


def _get_nc():
    if "nc" not in _NC_CACHE:
        _NC_CACHE["nc"] = build_program()
    return _NC_CACHE["nc"]


def kernel(**inputs):
    inputs = {k: np.asarray(v) for k, v in inputs.items()}
    nc = _get_nc()
    in_maps = host_prep(inputs)
    res = bass_utils.run_bass_kernel_spmd(nc, in_maps,
                                          core_ids=list(range(NCORES)))
    outs = []
    uvs = []
    for core in range(NCORES):
        r = res.results[core]
        outs.append(r["out_d"])                              # [128, 1250]
        cu = []
        for nmkey in ("u_d", "v_d", "s_d"):
            a = r[nmkey].reshape(128, NT, 6)
            cu.append(np.transpose(a, (2, 1, 0)).reshape(6, NT * 128)[:, :10000])
        uvs.append(np.stack(cu, axis=-1))                    # [6, 10000, 3]
    out = np.concatenate(outs, axis=1).reshape(1, 128, 100, 100)
    cam_uv = np.concatenate(uvs, axis=1).reshape(1, 6, QG, 1, 8, 3)
    return out.astype(np.float32), cam_uv.astype(np.float32)


if __name__ == "__main__":
    d = np.load("/root/problem/ref_inputs.npz")
    inputs = {k: d[k] for k in d.files}
    out, cam_uv = kernel(**inputs)
    ref_out = np.load("/root/problem/ref_out.npy")
    ref_cu = np.load("/root/problem/ref_camuv.npy")
    def rel(a, b):
        return np.abs(a - b).max() / (np.abs(b).max() + 1e-9)
    print("out absmax-rel:", rel(out, ref_out))
    print("cam_uv absmax-rel:", rel(cam_uv, ref_cu))
    print("out l2rel:", np.linalg.norm(out - ref_out) / np.linalg.norm(ref_out))


# revision 23
# speedup vs baseline: 1.2506x; 1.2506x over previous
"""BEVSampling Trainium2 kernel.

Strategy: data-parallel over the Q=10000 BEV queries across 8 cores
(1250 queries = 10368 padded points per core). Per core, on device:
  - denormalize reference points + offsets (T4), project into all 6
    cameras with one small matmul per 128-point tile,
  - compute u/v/valid/bilinear weights/patch indices with wide DVE ops,
  - gather 2x2 bilinear patches (one 1KB descriptor per (cam,point))
    from a host-prepped pixel-major fp16 feature table via indirect DMA,
  - weighted tap-combine on DVE, PE-transpose to feature-major,
  - positional MLP + 3-layer per-query MLP entirely on TensorE,
  - outputs: out[C, 1250] fp32 and u/v/valid dumps for cam_uv.
Host only shards/relayouts inputs and concatenates outputs.
"""
import numpy as np

import concourse.bass as bass
import concourse.mybir as mybir
import concourse.tile as tile
from concourse import bass_utils
from concourse.masks import make_identity
from concourse.vector_clock import ScopedClock

F32 = mybir.dt.float32
F32R = mybir.dt.float32r
F16 = mybir.dt.float16
I32 = mybir.dt.int32
AOT = mybir.AluOpType
ACTF = mybir.ActivationFunctionType
AXT = mybir.AxisListType

PC_LO = np.array([-50.0, -50.0, -5.0], dtype=np.float32)
PC_SPAN = np.array([100.0, 100.0, 8.0], dtype=np.float32)
EPS = 1e-5
N, C, Hf, Wf, P = 6, 128, 28, 60, 8
NCORES = 8
QG = 10000                 # global queries
QC = 1250                  # real queries per core
MQ = 1296                  # padded queries per core (27*48)
M = MQ * 8                 # 10368 points per core
NT = M // 128              # 81 tiles of 128 points
NG, GM = 27, 384           # groups for the (g,j)-partition layout
R2 = N * Hf * Wf + 61      # table rows (padded)
NQCH = 3                   # query chunks in conv MLP
QCH = MQ // NQCH           # 432
TB = 3                     # m-tiles per indirect-DMA batch
NS = 3                     # camera slots per point after compaction


def _patch_tile_drain():
    """walrus in this container allows only 1 sync wait per Drain: spread
    the Tile epilogue drain's waits across multiple drain instructions."""
    if getattr(tile.TileContext, "_drain_patched", False):
        return

    def _drain_and_barrier(self, tick_clock, wait_clock):
        drain_inst = self.nc.sync.drain()
        wait_clock.add_sem_waits(
            drain_inst.ins, ScopedClock({None: tick_clock.global_clock})
        )
        si = drain_inst.ins.sync_info
        w = list(si.on_wait or []) if si is not None else []
        if len(w) > 1:
            si.on_wait = w[:1]
            for sw in w[1:]:
                d2 = self.nc.sync.drain()
                d2.ins.sync_info = mybir.SyncInfo(on_wait=[sw], on_update=[])
        self.nc.all_engine_barrier()
        assert self.sems is not None
        popped = self.nc._tile_sem_poison_stack.pop()
        assert popped is self._sem_poison
        self.nc.clear_and_free_semaphores(list(self.sems.allocated().values()))
        self.nc.all_engine_barrier()

    tile.TileContext._drain_and_barrier = _drain_and_barrier
    tile.TileContext._drain_patched = True


def _patch_compile_bir():
    """This walrus build accepts at most ONE sync wait per instruction.
    Tile emits several. Rewrite the serialized BIR before compiling:
    hoist extra waits onto same-engine NoOps inserted just before."""
    import json
    import concourse.bass2jax as b2j
    if getattr(b2j, "_split_wait_patched", False):
        return
    orig = b2j.compile_bir_kernel

    def _split(bir_bytes):
        j = json.loads(bir_bytes)
        ctr = [0]
        for fn in j.get("functions", []):
            for blk in fn.get("blocks", []):
                insts = blk.get("instructions", [])
                out = []
                for inst in insts:
                    si = inst.get("sync_info")
                    if si and si.get("on_wait") and len(si["on_wait"]) > 1:
                        waits = si["on_wait"]
                        for wv in waits[:-1]:
                            ctr[0] += 1
                            out.append({
                                "opcode": "NoOp",
                                "engine": inst.get("engine", "SP"),
                                "name": f"nopw-{ctr[0]}",
                                "ins": [],
                                "outs": [],
                                "debug": inst.get("debug", 0),
                                "sync_info": {"on_update": [],
                                              "on_wait": [wv]},
                            })
                        si["on_wait"] = [waits[-1]]
                    out.append(inst)
                blk["instructions"] = out
        return json.dumps(j).encode()

    def wrapped(bir_bytes, *a, **k):
        return orig(_split(bir_bytes), *a, **k)

    b2j.compile_bir_kernel = wrapped
    b2j._split_wait_patched = True


def build_program():
    _patch_tile_drain()
    _patch_compile_bir()
    nc = bass.Bass("TRN2", target_bir_lowering=False)

    di = {}
    def inp(name, shape, dt):
        di[name] = nc.dram_tensor(name, shape, dt, kind="ExternalInput")
        return di[name]

    rp_w4 = inp("rp_w4", [128, GM], F32)
    off_w4 = inp("off_w4", [128, GM], F32)
    scale_vec = inp("scale_vec", [128, 1], F32)
    bias_vec = inp("bias_vec", [128, 1], F32)
    s2_vec = inp("s2_vec", [128, 1], F32)
    b2n_vec = inp("b2n_vec", [128, 1], F32)
    L_in = inp("L_in", [4, 24], F32)
    T2 = inp("T2", [R2, 512], F16)
    w1_in = inp("w1_in", [3, 256], F16)
    b1_in = inp("b1_in", [128, 2], F32)
    w2_in = inp("w2_in", [128, 2, 128], F16)
    b2_in = inp("b2_in", [128, 1], F32)
    cw1_in = inp("cw1_in", [128, 8, 512], F16)
    cb1_in = inp("cb1_in", [128, 4], F32)
    cw2_in = inp("cw2_in", [128, 4, 512], F16)
    cb2_in = inp("cb2_in", [128, 4], F32)
    cw3_in = inp("cw3_in", [128, 4, 128], F16)
    cb3_in = inp("cb3_in", [128, 1], F32)

    t4_dram = nc.dram_tensor("t4_dram", [128, GM], F32)
    t4n_dram = nc.dram_tensor("t4n_dram", [128, GM], F32)
    out_d = nc.dram_tensor("out_d", [128, QC], F32, kind="ExternalOutput")
    u_d = nc.dram_tensor("u_d", [128, NT * 6], F32, kind="ExternalOutput")
    v_d = nc.dram_tensor("v_d", [128, NT * 6], F32, kind="ExternalOutput")
    s_d = nc.dram_tensor("s_d", [128, NT * 6], F32, kind="ExternalOutput")

    W = NT * 6  # 486

    with tile.TileContext(nc) as tc:
        with tc.tile_pool(name="const", bufs=1) as cpool, \
             tc.tile_pool(name="keep", bufs=1) as kpool, \
             tc.tile_pool(name="psA", bufs=2, space="PSUM") as psA, \
             tc.tile_pool(name="psB", bufs=2, space="PSUM") as psB:

            # ---- constant / input loads ----
            def load(dram, shape, dt, name):
                t = cpool.tile(shape, dt, tag=name, name=name)
                nc.sync.dma_start(t[:], dram[:])
                return t
            scv = load(scale_vec, [128, 1], F32, "scv")
            biv = load(bias_vec, [128, 1], F32, "biv")
            s2v = load(s2_vec, [128, 1], F32, "s2v")
            b2nv = load(b2n_vec, [128, 1], F32, "b2nv")
            Lsb = load(L_in, [4, 24], F32, "Lsb")
            w1s = load(w1_in, [3, 256], F16, "w1s")
            b1s = load(b1_in, [128, 2], F32, "b1s")
            w2s = load(w2_in, [128, 2, 128], F16, "w2s")
            b2s = load(b2_in, [128, 1], F32, "b2s")
            cw1s = load(cw1_in, [128, 8, 512], F16, "cw1s")
            cb1s = load(cb1_in, [128, 4], F32, "cb1s")
            cw2s = load(cw2_in, [128, 4, 512], F16, "cw2s")
            cb2s = load(cb2_in, [128, 4], F32, "cb2s")
            cw3s = load(cw3_in, [128, 4, 128], F16, "cw3s")
            cb3s = load(cb3_in, [128, 1], F32, "cb3s")

            ident = cpool.tile([128, 128], F16, tag="ident")
            make_identity(nc, ident[:])

            # persistent across phases
            xT = kpool.tile([128, M], F16, tag="xT")
            W4h = kpool.tile([128, NT, NS, 4], F16, tag="W4h")
            idx32 = kpool.tile([128, NT, NS], I32, tag="idx32")
            outs = kpool.tile([128, MQ], F32, tag="outs")

            CH = 1152               # m per chunk (3 groups, 9 m-tiles)
            NCH = M // CH           # 9

            with tc.tile_pool(name="ph1", bufs=1) as wpool, \
                 tc.tile_pool(name="ph1d", bufs=2) as dpool:
                rp4 = wpool.tile([128, GM], F32, tag="rp4")
                off4 = wpool.tile([128, GM], F32, tag="off4")
                nc.sync.dma_start(rp4[:], rp_w4[:])
                nc.sync.dma_start(off4[:], off_w4[:])
                iota_n = wpool.tile([128, NT, 6], F32, tag="iota_n")
                nc.gpsimd.iota(iota_n[:], pattern=[[0, NT], [1680, 6]], base=0,
                               channel_multiplier=0,
                               allow_small_or_imprecise_dtypes=True)

                # Phase A: T4 (world homog points), T4n (renormalized)
                T4 = wpool.tile([128, GM], F32, tag="T4")
                nc.scalar.activation(T4[:], rp4[:], ACTF.Identity,
                                     bias=biv[:, :1], scale=scv[:, :1])
                nc.vector.tensor_add(T4[:], T4[:], off4[:])
                T4n = wpool.tile([128, GM], F32, tag="T4n")
                nc.scalar.activation(T4n[:], T4[:], ACTF.Identity,
                                     bias=b2nv[:, :1], scale=s2v[:, :1])
                nc.sync.dma_start(t4_dram[:], T4[:])
                nc.sync.dma_start(t4n_dram[:], T4n[:])

                # per-chunk: shuffle to [4, CH], project, pos-MLP into xT
                proj = wpool.tile([128, NT, 24], F32, tag="proj")
                for c in range(NCH):
                    g0 = c * 3
                    t4c = dpool.tile([4, CH], F32, tag="t4c")
                    t4nc = dpool.tile([4, CH], F16, tag="t4nc")
                    src = t4_dram[g0 * 4:(g0 + 3) * 4, :].rearrange(
                        "(g j) mm -> j g mm", j=4)
                    nc.sync.dma_start(
                        t4c[:].rearrange("j (g mm) -> j g mm", g=3), src)
                    srcn = t4n_dram[g0 * 4:(g0 + 3) * 4, :].rearrange(
                        "(g j) mm -> j g mm", j=4)
                    nc.gpsimd.dma_start(
                        t4nc[:].rearrange("j (g mm) -> j g mm", g=3), srcn)

                    # projection: 9 m-tiles -> psum -> proj
                    pst = psA.tile([128, 216], F32, tag="pp")
                    for tt in range(9):
                        t = c * 9 + tt
                        nc.tensor.matmul(
                            pst[:, tt * 24:(tt + 1) * 24],
                            t4c[:, tt * 128:(tt + 1) * 128],
                            Lsb[:4, :],
                            start=True, stop=True)
                    nc.vector.tensor_copy(proj[:, c * 9:(c + 1) * 9, :], pst[:])

                    # pos MLP for the 3 groups of this chunk -> xT
                    for gg in range(3):
                        rhs = t4nc[:3, gg * GM:(gg + 1) * GM]
                        h1g = dpool.tile([128, 2, GM], F16, tag="h1g")
                        for h in range(2):
                            psh = psB.tile([128, GM], F32, tag="psh")
                            nc.tensor.matmul(
                                psh[:],
                                w1s[:3, h * 128:(h + 1) * 128],
                                rhs,
                                start=True, stop=True)
                            nc.scalar.activation(h1g[:, h], psh[:], ACTF.Relu,
                                                 bias=b1s[:, h:h + 1], scale=1.0)
                        ppos = psB.tile([128, GM], F32, tag="ppos")
                        for h in range(2):
                            nc.tensor.matmul(ppos[:], w2s[:, h], h1g[:, h],
                                             start=(h == 0), stop=(h == 1))
                        g = g0 + gg
                        nc.vector.tensor_copy(xT[:, g * GM:(g + 1) * GM],
                                              ppos[:])

                # Phase B2: u/v/valid/weights/indices (wide ops over all tiles)
                xv = proj[:, :, 0:24:4]
                yv = proj[:, :, 1:24:4]
                zv = proj[:, :, 2:24:4]
                def wt(name, dt=F32):
                    return wpool.tile([128, NT, 6], dt, tag=name, name=name)
                zd = wt("zd"); r0 = wt("r0"); e = wt("e"); r = wt("r")
                xr = wt("xr"); yr = wt("yr")
                nc.vector.tensor_scalar_max(zd[:], zv, EPS)
                nc.vector.reciprocal(r0[:], zd[:])
                nc.vector.tensor_mul(e[:], zd[:], r0[:])
                nc.scalar.activation(e[:], e[:], ACTF.Copy, bias=2.0, scale=-1.0)
                nc.vector.tensor_mul(r[:], r0[:], e[:])
                nc.vector.tensor_mul(xr[:], xv, r[:])
                nc.vector.tensor_mul(yr[:], yv, r[:])

                Usb = wpool.tile([128, NT, 6], F32, tag="Usb")
                Vsb = wpool.tile([128, NT, 6], F32, tag="Vsb")
                Ssb = wpool.tile([128, NT, 6], F32, tag="Ssb")
                nc.vector.tensor_scalar_mul(Usb[:], xr[:], 1.0 / 480.0)
                nc.vector.tensor_scalar_mul(Vsb[:], yr[:], 1.0 / 224.0)

                xi = wt("xi"); yi = wt("yi")
                nc.scalar.activation(xi[:], xr[:], ACTF.Copy, bias=-0.5, scale=0.125)
                nc.scalar.activation(yi[:], yr[:], ACTF.Copy, bias=-0.5, scale=0.125)

                ta = wt("ta"); tb = wt("tb"); tcm = wt("tcm")
                nc.vector.tensor_scalar(ta[:], zv, EPS, None, op0=AOT.is_gt)
                nc.vector.tensor_scalar(tb[:], Usb[:], 0.0, None, op0=AOT.is_gt)
                nc.vector.tensor_scalar(tcm[:], Usb[:], 1.0, None, op0=AOT.is_lt)
                nc.vector.tensor_mul(ta[:], ta[:], tb[:])
                nc.vector.tensor_mul(ta[:], ta[:], tcm[:])
                nc.vector.tensor_scalar(tb[:], Vsb[:], 0.0, None, op0=AOT.is_gt)
                nc.vector.tensor_scalar(tcm[:], Vsb[:], 1.0, None, op0=AOT.is_lt)
                nc.vector.tensor_mul(ta[:], ta[:], tb[:])
                nc.vector.tensor_mul(Ssb[:], ta[:], tcm[:])      # valid

                # floor(xi) -> x0 ; floor(yi) -> y0
                ic = wpool.tile([128, NT, 6], I32, tag="ic")
                x0 = wt("x0"); y0 = wt("y0"); wx = wt("wx"); wy = wt("wy")
                nc.vector.tensor_copy(ic[:], xi[:])
                nc.vector.tensor_copy(x0[:], ic[:])
                nc.vector.tensor_tensor(tb[:], x0[:], xi[:], op=AOT.is_gt)
                nc.vector.tensor_sub(x0[:], x0[:], tb[:])
                nc.vector.tensor_copy(ic[:], yi[:])
                nc.vector.tensor_copy(y0[:], ic[:])
                nc.vector.tensor_tensor(tb[:], y0[:], yi[:], op=AOT.is_gt)
                nc.vector.tensor_sub(y0[:], y0[:], tb[:])
                nc.vector.tensor_sub(wx[:], xi[:], x0[:])
                nc.vector.tensor_sub(wy[:], yi[:], y0[:])

                # a0/a1 (x taps), b0/b1 (y taps)
                a0 = wt("a0"); a1 = wt("a1"); b0 = wt("b0"); b1 = wt("b1")
                omw = wt("omw")
                nc.vector.tensor_scalar(ta[:], x0[:], 0.0, None, op0=AOT.is_ge)
                nc.vector.tensor_scalar(tb[:], x0[:], 59.0, None, op0=AOT.is_le)
                nc.vector.tensor_mul(tb[:], ta[:], tb[:])
                nc.scalar.activation(omw[:], wx[:], ACTF.Copy, bias=1.0, scale=-1.0)
                nc.vector.tensor_mul(a0[:], tb[:], omw[:])
                nc.vector.tensor_scalar(tcm[:], x0[:], -1.0, None, op0=AOT.is_equal)
                nc.vector.tensor_mul(tcm[:], tcm[:], wx[:])
                nc.vector.tensor_add(a0[:], a0[:], tcm[:])
                nc.vector.tensor_scalar(tb[:], x0[:], 58.0, None, op0=AOT.is_le)
                nc.vector.tensor_mul(tb[:], ta[:], tb[:])
                nc.vector.tensor_mul(a1[:], tb[:], wx[:])
                nc.vector.tensor_scalar(ta[:], y0[:], 0.0, None, op0=AOT.is_ge)
                nc.vector.tensor_scalar(tb[:], y0[:], 27.0, None, op0=AOT.is_le)
                nc.vector.tensor_mul(tb[:], ta[:], tb[:])
                nc.scalar.activation(omw[:], wy[:], ACTF.Copy, bias=1.0, scale=-1.0)
                nc.vector.tensor_mul(b0[:], tb[:], omw[:])
                nc.vector.tensor_scalar(tcm[:], y0[:], -1.0, None, op0=AOT.is_equal)
                nc.vector.tensor_mul(tcm[:], tcm[:], wy[:])
                nc.vector.tensor_add(b0[:], b0[:], tcm[:])
                nc.vector.tensor_scalar(tb[:], y0[:], 26.0, None, op0=AOT.is_le)
                nc.vector.tensor_mul(tb[:], ta[:], tb[:])
                nc.vector.tensor_mul(b1[:], tb[:], wy[:])
                nc.vector.tensor_mul(a0[:], a0[:], Ssb[:])
                nc.vector.tensor_mul(a1[:], a1[:], Ssb[:])

                # weights fp16, k = (b0a0, b1a0, b0a1, b1a1)
                W4f = wpool.tile([128, NT, 6, 4], F32, tag="W4f")
                nc.vector.tensor_mul(W4f[:, :, :, 0], b0[:], a0[:])
                nc.vector.tensor_mul(W4f[:, :, :, 1], b0[:], a1[:])
                nc.vector.tensor_mul(W4f[:, :, :, 2], b1[:], a0[:])
                nc.vector.tensor_mul(W4f[:, :, :, 3], b1[:], a1[:])
                # patch index = clip(y0,0,27)*60 + clip(x0,0,59) + n*1680
                nc.vector.tensor_scalar(ta[:], x0[:], 0.0, 59.0,
                                        op0=AOT.max, op1=AOT.min)
                nc.vector.tensor_scalar(tb[:], y0[:], 0.0, 27.0,
                                        op0=AOT.max, op1=AOT.min)
                nc.vector.tensor_scalar_mul(tb[:], tb[:], 60.0)
                nc.vector.tensor_add(ta[:], ta[:], tb[:])
                nc.vector.tensor_add(ta[:], ta[:], iota_n[:])

                # per-point camera compaction into NS slots
                cum = wpool.tile([128, NT, 6], F32, tag="cum")
                nc.vector.memset(cum[:, :, 0], 0.0)
                for n in range(1, 6):
                    nc.vector.tensor_add(cum[:, :, n], cum[:, :, n - 1],
                                         Ssb[:, :, n - 1])
                sel = wpool.tile([128, NT, 6], F32, tag="sel")
                msk = wpool.tile([128, NT, 6], F32, tag="msk")
                red = wpool.tile([128, NT], F32, tag="red")
                W4S = wpool.tile([128, NT, NS, 4], F32, tag="W4S")
                for ss in range(NS):
                    nc.vector.tensor_scalar(sel[:], cum[:], float(ss), None,
                                            op0=AOT.is_equal)
                    nc.vector.tensor_mul(sel[:], sel[:], Ssb[:])
                    nc.vector.tensor_mul(msk[:], sel[:], ta[:])
                    nc.vector.tensor_reduce(red[:], msk[:], axis=AXT.X,
                                            op=AOT.add)
                    nc.vector.tensor_copy(idx32[:, :, ss], red[:])
                    for k in range(4):
                        nc.vector.tensor_mul(msk[:], sel[:], W4f[:, :, :, k])
                        nc.vector.tensor_reduce(W4S[:, :, ss, k], msk[:],
                                                axis=AXT.X, op=AOT.add)
                nc.vector.tensor_copy(W4h[:], W4S[:])

                nc.sync.dma_start(u_d[:],
                                  Usb[:].rearrange("p t n -> p (t n)"))
                nc.sync.dma_start(v_d[:],
                                  Vsb[:].rearrange("p t n -> p (t n)"))
                nc.sync.dma_start(s_d[:],
                                  Ssb[:].rearrange("p t n -> p (t n)"))

            # ---- Phase C+D: gather + combine + transpose (adds into xT) ----
            with tc.tile_pool(name="gbuf", bufs=3) as gpool, \
                 tc.tile_pool(name="small", bufs=2) as spool:
                for t in range(NT):
                    G4 = gpool.tile([128, NS, 4, 128], F16, tag="G4")
                    for ss in range(NS):
                        nc.gpsimd.indirect_dma_start(
                            out=G4[:, ss].rearrange("p a b -> p (a b)"),
                            out_offset=None,
                            in_=T2[:, :],
                            in_offset=bass.IndirectOffsetOnAxis(
                                ap=idx32[:, t, ss:ss + 1], axis=0),
                        )
                    Gw = spool.tile([128, NS * 4, 128], F16, tag="Gw")
                    wbc = W4h[:, t].rearrange(
                        "p n k -> p (n k)").to_broadcast([128, NS * 4, 128])
                    nc.vector.tensor_tensor(
                        Gw[:], G4[:].rearrange("p n k c -> p (n k) c"), wbc,
                        op=AOT.mult)
                    r1 = spool.tile([128, 6, 128], F16, tag="r1")
                    nc.vector.tensor_add(r1[:], Gw[:, 0:6], Gw[:, 6:12])
                    r2 = spool.tile([128, 3, 128], F16, tag="r2")
                    nc.vector.tensor_add(r2[:], r1[:, 0:3], r1[:, 3:6])
                    samp = spool.tile([128, 128], F16, tag="samp")
                    nc.vector.tensor_add(samp[:], r2[:, 0], r2[:, 1])
                    nc.vector.tensor_add(samp[:], samp[:], r2[:, 2])
                    ptr = psA.tile([128, 128], F16, tag="pp")
                    nc.tensor.transpose(ptr[:], samp[:], ident[:])
                    xs = xT[:, t * 128:(t + 1) * 128]
                    nc.vector.tensor_add(xs, xs, ptr[:])

            # + b2 (per-channel)
            nc.vector.tensor_scalar_add(xT[:], xT[:], b2s[:, :1])

            # ---- Phase F: conv MLP ----
            with tc.tile_pool(name="mlp", bufs=2) as mpool:
                xTv = xT.rearrange("c (q p) -> c q p", p=8)
                for qc in range(NQCH):
                    q0 = qc * QCH
                    h1c = mpool.tile([128, 4, QCH], F16, tag="h1c")
                    for mt in range(4):
                        ph = psA.tile([128, QCH], F32, tag="ph")
                        for p in range(8):
                            nc.tensor.matmul(
                                ph[:], cw1s[:, p, mt * 128:(mt + 1) * 128],
                                xTv[:, q0:q0 + QCH, p],
                                start=(p == 0), stop=(p == 7))
                        nc.scalar.activation(h1c[:, mt], ph[:], ACTF.Gelu,
                                             bias=cb1s[:, mt:mt + 1], scale=1.0)
                    h2c = mpool.tile([128, 4, QCH], F16, tag="h2c")
                    for mt in range(4):
                        ph = psA.tile([128, QCH], F32, tag="ph")
                        for k in range(4):
                            nc.tensor.matmul(
                                ph[:], cw2s[:, k, mt * 128:(mt + 1) * 128],
                                h1c[:, k],
                                start=(k == 0), stop=(k == 3))
                        nc.scalar.activation(h2c[:, mt], ph[:], ACTF.Gelu,
                                             bias=cb2s[:, mt:mt + 1], scale=1.0)
                    ph = psA.tile([128, QCH], F32, tag="ph")
                    for k in range(4):
                        nc.tensor.matmul(ph[:], cw3s[:, k], h2c[:, k],
                                         start=(k == 0), stop=(k == 3))
                    nc.scalar.activation(outs[:, q0:q0 + QCH], ph[:],
                                         ACTF.Identity, bias=cb3s[:, :1],
                                         scale=1.0)

            nc.sync.dma_start(out_d[:], outs[:, :QC])

    return nc


def host_prep(inputs):
    """Build the 8 per-core input dicts (pure relayout / fp16 cast)."""
    rp = np.ascontiguousarray(inputs["reference_points"][0].reshape(QG, 3))
    off = np.ascontiguousarray(inputs["sampling_offset"][0].reshape(QG, 8, 3))
    feats = inputs["feats"][0]
    l2i = inputs["lidar2img"][0]

    # shared tensors
    scale_vec = np.zeros((128, 1), np.float32)
    bias_vec = np.zeros((128, 1), np.float32)
    s2_vec = np.zeros((128, 1), np.float32)
    b2n_vec = np.zeros((128, 1), np.float32)
    for g in range(NG):
        for j in range(3):
            r = g * 4 + j
            scale_vec[r] = PC_SPAN[j]
            bias_vec[r] = PC_LO[j]
            s2_vec[r] = np.float32(1.0) / PC_SPAN[j]
            b2n_vec[r] = -PC_LO[j] / PC_SPAN[j]
        scale_vec[g * 4 + 3] = 0.0
        bias_vec[g * 4 + 3] = 1.0

    L_in = np.zeros((4, 24), np.float32)
    for n in range(N):
        for i in range(4):
            L_in[:, n * 4 + i] = l2i[n, i, :]

    T2 = np.zeros((R2, 512), np.float16)
    fpix = feats.astype(np.float16).transpose(0, 2, 3, 1)    # [N,Hf,Wf,C]
    ent = np.zeros((N, Hf, Wf, 4, 128), np.float16)
    ent[:, :, :, 0] = fpix
    ent[:, :, :Wf - 1, 1] = fpix[:, :, 1:]
    ent[:, :Hf - 1, :, 2] = fpix[:, 1:, :]
    ent[:, :Hf - 1, :Wf - 1, 3] = fpix[:, 1:, 1:]
    T2[:N * Hf * Wf] = ent.reshape(-1, 512)

    w1_in = np.ascontiguousarray(inputs["w1"]).astype(np.float16)
    b1_in = np.zeros((128, 2), np.float32)
    b1_in[:, 0] = inputs["b1"][:128]
    b1_in[:, 1] = inputs["b1"][128:]
    w2_in = np.ascontiguousarray(
        inputs["w2"].reshape(2, 128, 128).transpose(1, 0, 2)).astype(np.float16)
    b2_in = np.ascontiguousarray(inputs["b2"].reshape(128, 1)).astype(np.float32)
    cw1_in = np.ascontiguousarray(
        inputs["cw1"].reshape(8, 128, 512).transpose(1, 0, 2)).astype(np.float16)
    cb1_in = np.ascontiguousarray(
        inputs["cb1"].reshape(4, 128).T).astype(np.float32)
    cw2_in = np.ascontiguousarray(
        inputs["cw2"].reshape(4, 128, 512).transpose(1, 0, 2)).astype(np.float16)
    cb2_in = np.ascontiguousarray(
        inputs["cb2"].reshape(4, 128).T).astype(np.float32)
    cw3_in = np.ascontiguousarray(
        inputs["cw3"].reshape(4, 128, 128).transpose(1, 0, 2)).astype(np.float16)
    cb3_in = np.ascontiguousarray(inputs["cb3"].reshape(128, 1)).astype(np.float32)

    shared = dict(scale_vec=scale_vec, bias_vec=bias_vec, s2_vec=s2_vec,
                  b2n_vec=b2n_vec, L_in=L_in, T2=T2, w1_in=w1_in, b1_in=b1_in,
                  w2_in=w2_in, b2_in=b2_in, cw1_in=cw1_in, cb1_in=cb1_in,
                  cw2_in=cw2_in, cb2_in=cb2_in, cw3_in=cw3_in, cb3_in=cb3_in)

    in_maps = []
    marr = np.arange(M)
    for core in range(NCORES):
        qidx = np.minimum(core * QC + marr // 8, QG - 1)
        pidx = marr % 8
        rp_w4 = np.zeros((128, GM), np.float32)
        off_w4 = np.zeros((128, GM), np.float32)
        rpm = rp[qidx]                                       # [M, 3]
        offm = off[qidx, pidx]                               # [M, 3]
        rpv = rpm.reshape(NG, GM, 3)
        offv = offm.reshape(NG, GM, 3)
        for g in range(NG):
            for j in range(3):
                rp_w4[g * 4 + j, :] = rpv[g, :, j]
                off_w4[g * 4 + j, :] = offv[g, :, j]
        m = dict(shared)
        m["rp_w4"] = rp_w4
        m["off_w4"] = off_w4
        in_maps.append(m)
    return in_maps


_NC_CACHE = {}


def _get_nc():
    if "nc" not in _NC_CACHE:
        _NC_CACHE["nc"] = build_program()
    return _NC_CACHE["nc"]


def kernel(**inputs):
    inputs = {k: np.asarray(v) for k, v in inputs.items()}
    nc = _get_nc()
    in_maps = host_prep(inputs)
    res = bass_utils.run_bass_kernel_spmd(nc, in_maps,
                                          core_ids=list(range(NCORES)))
    outs = []
    uvs = []
    for core in range(NCORES):
        r = res.results[core]
        outs.append(r["out_d"])                              # [128, 1250]
        cu = []
        for nmkey in ("u_d", "v_d", "s_d"):
            a = r[nmkey].reshape(128, NT, 6)
            cu.append(np.transpose(a, (2, 1, 0)).reshape(6, NT * 128)[:, :10000])
        uvs.append(np.stack(cu, axis=-1))                    # [6, 10000, 3]
    out = np.concatenate(outs, axis=1).reshape(1, 128, 100, 100)
    cam_uv = np.concatenate(uvs, axis=1).reshape(1, 6, QG, 1, 8, 3)
    return out.astype(np.float32), cam_uv.astype(np.float32)


if __name__ == "__main__":
    d = np.load("/root/problem/ref_inputs.npz")
    inputs = {k: d[k] for k in d.files}
    out, cam_uv = kernel(**inputs)
    ref_out = np.load("/root/problem/ref_out.npy")
    ref_cu = np.load("/root/problem/ref_camuv.npy")
    def rel(a, b):
        return np.abs(a - b).max() / (np.abs(b).max() + 1e-9)
    print("out absmax-rel:", rel(out, ref_out))
    print("cam_uv absmax-rel:", rel(cam_uv, ref_cu))
    print("out l2rel:", np.linalg.norm(out - ref_out) / np.linalg.norm(ref_out))
